# revision 36
# baseline (speedup 1.0000x reference)
"""Trainium2 Bass kernel v5: 16-filter binarized 5x5 VALID conv.

x [32, 6, 512, 512] f32 -> out [32, 16, 508, 508] f32, data-parallel over
batch on 8 cores (4 images/core).

Per-core mapping (v5: single-row slabs, row-parity folded into weights):

  Row groups: 16 output rows per main group (31 groups + one 12-row tail
  per image). SBUF slot per group: [120, 512] bf16 = 6 channels x 20
  window rows, ONE image row per partition, partition index j*6+c
  (j = row-in-window). Loaded with no overlap (1.25x halo only) as two
  HWDGE DMAs per group (dim0 16 + 4 -> spreads across all 16 SDMA
  engines; a 3D DMA AP fans out by dim0 and needs dim0 <= 16).

  Matmul: K = 120 (all window rows), M = 128 = (o in 16) x (rb in 8),
  N = 508. The SAME moving AP serves both row parities: weight block
  (dx, rp) has nonzeros at K row (j, c) iff j = 2*rb + rp + dx-tap dy.
  10 accumulating matmuls per group (dx 0..4 x rp 0..1) into two PSUM
  tiles [128, 508] f32 (one bank each).

  Evacuation: DVE (tensor_scalar_mul by per-filter alpha, f32 -> bf16);
  the Scalar/Sync sequencers are reserved for DMA issue, whose ~0.65us
  fixed cost per dma_start is the issue-rate wall. Only the run's very
  last group splits its two evacs DVE||ACT to shorten the end tail.

  Stores: bf16, PSUM-native layout out_main [b, 128, 31*1016], one store
  per 4-group stage (8 KB descriptors; final chunk stages by 2 so the
  last store is one group); host un-permutes + casts f32.

  Lead-in: weight tables load first as single DMAs whose host-permuted
  rows + non-mergeable 3-level src AP fan across 15/12 engines; ~4us of
  full-K dummy matmuls warm the PE HAM clock gate to 2.4 GHz before the
  first real matmul.
"""

import numpy as np
import ml_dtypes

import concourse.bass as bass
import concourse.mybir as mybir
from concourse import bacc
from concourse import tile
from concourse.bass_utils import run_bass_kernel_spmd

MAPS3 = np.array([[0, 1, 2], [1, 2, 3], [2, 3, 4], [3, 4, 5], [0, 4, 5], [0, 1, 5]])
MAPS4 = np.array(
    [
        [0, 1, 2, 3],
        [1, 2, 3, 4],
        [2, 3, 4, 5],
        [0, 3, 4, 5],
        [0, 1, 4, 5],
        [0, 1, 2, 5],
        [0, 1, 3, 4],
        [1, 2, 4, 5],
        [0, 2, 3, 5],
    ]
)

C_IN = 6
N_OUT = 16
KH = KW = 5
RG = 16  # output rows per main group
NR = 20  # window rows per main group (16 + 4 halo)
K_MAIN = C_IN * NR  # 120
NRB = 8  # row-pairs per group
M_MAIN = N_OUT * NRB  # 128
NBLK = KW * 2  # weight blocks per group: (dx, rp)
R_TAIL = 12  # output rows in the tail group (496..507)
NR_T = 16  # tail window rows (496..511)
K_TAIL = C_IN * NR_T  # 96
NRB_T = 6
M_TAIL = N_OUT * NRB_T  # 96
N_CORES = 8
NG = 8  # groups per chunk tile (chunks: 8,8,8,7)
SG = 4  # groups per staged store


def _binarize_np(w):
    w = np.asarray(w, dtype=np.float32)
    m = w - w.mean(axis=1, keepdims=True)
    c = np.clip(m, -1.0, 1.0)
    alpha = np.abs(c).mean(axis=(1, 2, 3))
    return np.sign(c).astype(np.float32), alpha.astype(np.float32)


def _filter_table(w3, w4, w6):
    s3, a3 = _binarize_np(w3)
    s4, a4 = _binarize_np(w4)
    s6, a6 = _binarize_np(w6)
    table = []
    for o in range(6):
        table.append((list(MAPS3[o]), s3[o], a3[o]))
    for o in range(9):
        table.append((list(MAPS4[o]), s4[o], a4[o]))
    table.append((list(range(6)), s6[0], a6[0]))
    return table


def _build_weight_inputs(w3, w4, w6):
    """wm [120, 10*128], wmt [96, 10*96] bf16; al [128,1], alt [96,1] f32.

    K row index = j*6 + c (j = window row, c = channel). Weight block
    b = dx*2 + rp: column m=(o,rb) nonzero at j = 2*rb + rp + dy.
    """
    table = _filter_table(w3, w4, w6)
    wm = np.zeros((K_MAIN, NBLK * M_MAIN), dtype=np.float32)
    wmt = np.zeros((K_TAIL, NBLK * M_TAIL), dtype=np.float32)
    al = np.zeros((M_MAIN, 1), dtype=np.float32)
    alt = np.zeros((M_TAIL, 1), dtype=np.float32)
    for o, (chans, sgn, alpha) in enumerate(table):
        for rb in range(NRB):
            m = o * NRB + rb
            al[m, 0] = alpha
            for dx in range(KW):
                for rp in range(2):
                    b = dx * 2 + rp
                    for ci, c in enumerate(chans):
                        for dy in range(KH):
                            j = 2 * rb + rp + dy
                            wm[j * C_IN + c, b * M_MAIN + m] = sgn[ci, dy, dx]
        for rb in range(NRB_T):
            m = o * NRB_T + rb
            alt[m, 0] = alpha
            for dx in range(KW):
                for rp in range(2):
                    b = dx * 2 + rp
                    for ci, c in enumerate(chans):
                        for dy in range(KH):
                            j = 2 * rb + rp + dy
                            wmt[j * C_IN + c, b * M_TAIL + m] = sgn[ci, dy, dx]
    # permute rows so the load's non-mergeable 3-level src AP
    # [[F,15],[F*15,8],[1,F]] (which fans out over 15 SDMA engines)
    # lands row q of the original at SBUF partition q
    qm = np.arange(K_MAIN)
    wm_r = wm[(qm % 15) * 8 + qm // 15]
    qt = np.arange(K_TAIL)
    wmt_r = wmt[(qt % 12) * 8 + qt // 12]
    return (
        wm_r.astype(ml_dtypes.bfloat16),
        wmt_r.astype(ml_dtypes.bfloat16),
        al,
        alt,
    )


def build_nc(b_per_core, h, w, num_cores=N_CORES):
    h_out, w_out = h - KH + 1, w - KW + 1
    n_groups = (h_out - R_TAIL) // RG  # 31
    assert n_groups * RG + R_TAIL == h_out
    tail_start = h_out - R_TAIL  # 496
    NN = 2 * w_out  # out elems per group-row-pair slot (1016)
    f32 = mybir.dt.float32
    bf16 = mybir.dt.bfloat16

    chunks = []
    g0 = 0
    while g0 < n_groups:
        chunks.append((g0, min(NG, n_groups - g0)))
        g0 += NG

    nc = bacc.Bacc(
        "TRN2",
        target_bir_lowering=False,
        debug=False,
        num_devices=num_cores,
    )
    x_t = nc.dram_tensor("xb", [b_per_core, C_IN, h, w], bf16, kind="ExternalInput")
    wm_t = nc.dram_tensor("wm", [K_MAIN, NBLK * M_MAIN], bf16, kind="ExternalInput")
    wmt_t = nc.dram_tensor("wmt", [K_TAIL, NBLK * M_TAIL], bf16, kind="ExternalInput")
    al_t = nc.dram_tensor("al", [M_MAIN, 1], f32, kind="ExternalInput")
    alt_t = nc.dram_tensor("alt", [M_TAIL, 1], f32, kind="ExternalInput")
    om_t = nc.dram_tensor(
        "out_main", [b_per_core, M_MAIN, n_groups * NN], bf16, kind="ExternalOutput"
    )
    ot_t = nc.dram_tensor(
        "out_tail", [b_per_core, M_TAIL, NN], bf16, kind="ExternalOutput"
    )

    with tile.TileContext(nc) as tc:
        with (
            tc.tile_pool(name="wpool", bufs=1) as wpool,
            tc.tile_pool(name="xpool", bufs=6) as xpool,
            tc.tile_pool(name="tpool", bufs=2) as tpool,
            tc.tile_pool(name="spool", bufs=4) as spool,
            tc.tile_pool(name="s2pool", bufs=2) as s2pool,
            tc.tile_pool(name="ppool", bufs=3, space="PSUM") as ppool,
            tc.tile_pool(name="p2pool", bufs=1, space="PSUM") as p2pool,
        ):
            WM_F = NBLK * M_MAIN  # 1280
            wt = wpool.tile([K_MAIN, WM_F], bf16, tag="wt")
            WT_F = NBLK * M_TAIL  # 960
            wtt = wpool.tile([K_TAIL, WT_F], bf16, tag="wtt")
            at = wpool.tile([M_MAIN, 1], f32, tag="at")
            att = wpool.tile([M_TAIL, 1], f32, tag="att")

            def load_weights():
                # single dma each: host pre-permuted rows + non-mergeable
                # 3-level src AP -> balanced dim0=15/12 fans across engines
                # (dst stays a plain tile slice for safe addressing/deps)
                nc.sync.dma_start(
                    out=wt[:],
                    in_=bass.AP(
                        wm_t, 0, [[WM_F, 15], [WM_F * 15, 8], [1, WM_F]]
                    ),
                )
                nc.scalar.dma_start(
                    out=wtt[:],
                    in_=bass.AP(
                        wmt_t, 0, [[WT_F, 12], [WT_F * 12, 8], [1, WT_F]]
                    ),
                )
            def load_alphas():
                nc.sync.dma_start(out=at[:], in_=al_t[:])
                nc.scalar.dma_start(out=att[:], in_=alt_t[:])

            def load_chunk(b, ci):
                g0, ng = chunks[ci]
                xt = xpool.tile(
                    [K_MAIN, NG * w], bf16, tag="xt", name=f"xt_{b}_{ci}"
                )
                # one row per partition, partition index j*6+c; 3D DMA AP
                # fans out by dim0 (must be <=16): split j 20 -> 16 + 4
                for gl in range(ng):
                    base = b * C_IN * h * w + RG * (g0 + gl) * w
                    # split each group's dma pair across both sequencers so
                    # they issue in parallel (halves per-group issue latency)
                    eng_a, eng_b = (
                        (nc.sync, nc.scalar) if gl % 2 == 0 else (nc.scalar, nc.sync)
                    )
                    src_a = bass.AP(
                        x_t, base, [[w, 16], [h * w, C_IN], [1, w]]
                    )
                    eng_a.dma_start(
                        out=xt[0 : 16 * C_IN, gl * w : gl * w + w], in_=src_a
                    )
                    src_b = bass.AP(
                        x_t, base + 16 * w, [[w, 4], [h * w, C_IN], [1, w]]
                    )
                    eng_b.dma_start(
                        out=xt[16 * C_IN : K_MAIN, gl * w : gl * w + w],
                        in_=src_b,
                    )
                return xt

            def load_tail(b):
                xt2 = tpool.tile([K_TAIL, w], bf16, tag="xtt", name=f"xtt_{b}")
                src = bass.AP(
                    x_t,
                    b * C_IN * h * w + tail_start * w,
                    [[w, NR_T], [h * w, C_IN], [1, w]],
                )
                nc.sync.dma_start(out=xt2[:], in_=src)
                return xt2

            def do_group(xt, gl, stg, soff, split_evac=False):
                pss = [
                    ppool.tile([M_MAIN, w_out], f32, tag=f"ps{rp}", name=f"ps{rp}")
                    for rp in range(2)
                ]
                for dx in range(KW):
                    rhs = bass.AP(
                        xt[:].tensor,
                        gl * w + dx,
                        [[NG * w, K_MAIN], [1, w_out]],
                    )
                    for rp in range(2):
                        blk = dx * 2 + rp
                        nc.tensor.matmul(
                            pss[rp][:],
                            wt[:, blk * M_MAIN : (blk + 1) * M_MAIN],
                            rhs,
                            start=(dx == 0),
                            stop=(dx == KW - 1),
                        )
                for rp in range(2):
                    dst = stg[:, soff + rp * w_out : soff + (rp + 1) * w_out]
                    if split_evac and rp == 1:
                        # last group of the run: ACT in parallel with DVE
                        nc.scalar.mul(dst, pss[rp][:], at[:])
                    else:
                        nc.vector.tensor_scalar_mul(dst, pss[rp][:], at[:])

            def store_stage(b, gs, stg, ng_st):
                dst = bass.AP(
                    om_t,
                    b * M_MAIN * n_groups * NN + gs * NN,
                    [[n_groups * NN, M_MAIN], [1, ng_st * NN]],
                )
                nc.scalar.dma_start(out=dst, in_=stg[:, : ng_st * NN])

            def do_tail(b, xt2):
                pss = [
                    p2pool.tile([M_TAIL, w_out], f32, tag=f"pst{rp}", name=f"pst{rp}")
                    for rp in range(2)
                ]
                for dx in range(KW):
                    rhs = bass.AP(
                        xt2[:].tensor,
                        dx,
                        [[w, K_TAIL], [1, w_out]],
                    )
                    for rp in range(2):
                        blk = dx * 2 + rp
                        nc.tensor.matmul(
                            pss[rp][:],
                            wtt[:, blk * M_TAIL : (blk + 1) * M_TAIL],
                            rhs,
                            start=(dx == 0),
                            stop=(dx == KW - 1),
                        )
                st = s2pool.tile([M_TAIL, NN], bf16, tag="st2")
                for rp in range(2):
                    nc.vector.tensor_scalar_mul(
                        st[:, rp * w_out : (rp + 1) * w_out], pss[rp][:], att[:]
                    )
                dst = bass.AP(
                    ot_t, b * M_TAIL * NN, [[NN, M_TAIL], [1, NN]]
                )
                nc.scalar.dma_start(out=dst, in_=st[:])

            units = [(b, ci) for b in range(b_per_core) for ci in range(len(chunks))]
            LOOKAHEAD = 5
            xtiles = {}
            ttiles = {}
            # PE warmup: ~3us of dummy matmuls during the DMA lead-in flips
            # the HAM clock gate to 8/8 before the first real matmul
            # full-K/M footprint so the activity monitor actually counts it
            wsc = wpool.tile([K_MAIN, 320], bf16, tag="wsc")
            nc.vector.memset(wsc[:], 1.0)
            psc = ppool.tile([M_MAIN, w_out], f32, tag="ps0", name="psw")
            for i in range(20):
                nc.tensor.matmul(
                    psc[:, 0:256],
                    wsc[:, 0:M_MAIN],
                    wsc[:, 0:256],
                    start=(i == 0),
                    stop=(i == 19),
                )

            load_weights()  # 2 dmas, one per sequencer, ahead of chunk0
            for u in range(min(LOOKAHEAD, len(units))):
                b, ci = units[u]
                xtiles[(b, ci)] = load_chunk(b, ci)
                if ci == 0:
                    ttiles[b] = load_tail(b)
                if u == 0:
                    load_alphas()  # only needed at first evac, ~2us later

            for u, (b, ci) in enumerate(units):
                ul = u + LOOKAHEAD
                if ul < len(units):
                    bl, cl = units[ul]
                    xtiles[(bl, cl)] = load_chunk(bl, cl)
                    if cl == 0:
                        ttiles[bl] = load_tail(bl)
                xt = xtiles.pop((b, ci))
                g0, ng = chunks[ci]
                last_unit = u == len(units) - 1
                sg = 2 if last_unit else SG
                gl = 0
                while gl < ng:
                    ns = min(sg, ng - gl)
                    stg = spool.tile(
                        [M_MAIN, SG * NN], bf16, tag="stg",
                        name=f"stg_{b}_{ci}_{gl}",
                    )
                    for gg in range(ns):
                        do_group(
                            xt, gl + gg, stg, gg * NN,
                            split_evac=last_unit and gl + gg == ng - 1,
                        )
                    store_stage(b, g0 + gl, stg, ns)
                    gl += ns
                if ci == 0:
                    do_tail(b, ttiles.pop(b))

    nc.compile()
    return nc


_NC_CACHE = {}


def _get_nc(b_per_core, h, w):
    key = (b_per_core, h, w)
    if key not in _NC_CACHE:
        _NC_CACHE[key] = build_nc(b_per_core, h, w)
    return _NC_CACHE[key]


def _prep_inputs(x, w3, w4, w6):
    b = x.shape[0]
    assert b % N_CORES == 0
    bpc = b // N_CORES
    wm, wmt, al, alt = _build_weight_inputs(w3, w4, w6)
    xb = np.ascontiguousarray(x).astype(ml_dtypes.bfloat16)
    in_maps = [
        {
            "xb": np.ascontiguousarray(xb[i * bpc : (i + 1) * bpc]),
            "wm": wm,
            "wmt": wmt,
            "al": al,
            "alt": alt,
        }
        for i in range(N_CORES)
    ]
    return bpc, in_maps


def _unpermute(om, ot, bpc, h_out, w_out):
    """om [bpc, 128, 31*1016], ot [bpc, 96, 1016] -> [bpc, 16, 508, 508]."""
    n_groups = (h_out - R_TAIL) // RG
    out = np.empty((bpc, N_OUT, h_out, w_out), dtype=np.float32)
    m = om.reshape(bpc, N_OUT, NRB, n_groups, 2, w_out)
    out[:, :, : n_groups * RG] = m.transpose(0, 1, 3, 2, 4, 5).reshape(
        bpc, N_OUT, n_groups * RG, w_out
    )
    t = ot.reshape(bpc, N_OUT, NRB_T * 2, w_out)
    out[:, :, n_groups * RG :] = t
    return out


def run(x, w3, w4, w6, trace=False, **kw):
    b, c, h, w = x.shape
    h_out, w_out = h - 4, w - 4
    bpc, in_maps = _prep_inputs(x, w3, w4, w6)
    nc = _get_nc(bpc, h, w)
    res = run_bass_kernel_spmd(
        nc, in_maps, list(range(N_CORES)), trace=trace, **kw
    )
    outs = [
        _unpermute(
            np.asarray(r["out_main"], dtype=np.float32),
            np.asarray(r["out_tail"], dtype=np.float32),
            bpc, h_out, w_out,
        )
        for r in res.results
    ]
    return np.concatenate(outs, axis=0), res


def kernel(x, w3, w4, w6):
    out, _ = run(x, w3, w4, w6, trace=False)
    return out
